# revision 10
# baseline (speedup 1.0000x reference)
"""Trainium2 Bass kernel for nn_ActionPredictionModel (scatter_memory).

Data-parallel over graphs: 8 graphs (72 nodes) per NeuronCore, weights
replicated. Each core computes, for its graphs:
  - spec MLP (1801 -> 900 -> 100), via PE matmuls in transposed layout
  - value head (sum-pool readout + spec -> scalar)
  - pair action features, exploiting the block-diagonal structure: only
    the 9x9 same-graph pair blocks are materialized ([128ch, 648pairs])
  - per-graph flatten + indexmask gather (gpsimd ap_gather) + softmax
Host does only sharding/layout marshalling (transpose, pad, tile-pack,
index remap to the on-device fp layout) and output concatenation.
"""

import numpy as np

# problem dims (hardcoded per contract)
B, NPG, H = 64, 9, 128
SL, SC, BOND, ASL = 1801, 100, 3, 243
NCORES = 8
BPC = B // NCORES            # graphs per core = 8
NODES = BPC * NPG            # nodes per core = 72
PAIRS = BPC * NPG * NPG      # same-graph pairs per core = 648

KT = 15                      # k-tiles over spec dim (14*128 + 9)
HID = 900
HIDP = 904                   # padded hidden (8 * 113)
MCH = HIDP // 8              # hidden chunk = 113
SLP = KT * 128               # padded spec len = 1920

_CACHE = {}
DEBUG_TAPS = False


def _f32(x):
    return np.ascontiguousarray(np.asarray(x), dtype=np.float32)


def _build_nc():
    import concourse.mybir as mybir
    import concourse.tile as tile
    import concourse.bacc as bacc
    import concourse.bass as bass

    f32 = mybir.dt.float32
    i16 = mybir.dt.int16
    Alu = mybir.AluOpType
    Act = mybir.ActivationFunctionType

    nc = bacc.Bacc("TRN2", target_bir_lowering=False, debug=False, num_devices=1)

    # ---- dram params ----
    # consts layout (one [128, CF] f32 tensor), column offsets:
    #   wa2a[128] | wa2b[128] | wv1t[128] | wa2c[128] | wv2[1] | b2|bv1|bv2|ba2|bf (5)
    #   | wf[3] | B1t[64] | w2t[800]
    OFF_WA2A, OFF_WA2B, OFF_WV1 = 0, 128, 256
    OFF_WA2C, OFF_WV2 = 384, 512
    OFF_B2, OFF_BV1, OFF_BV2, OFF_BA2, OFF_BF = 513, 514, 515, 516, 517
    OFF_WF, OFF_B1T, OFF_W2T = 518, 521, 585
    CF = 585 + 800
    consts_d = nc.declare_dram_parameter("consts", [128, CF], f32, isOutput=False)
    acts_d = nc.declare_dram_parameter("acts", [128, 192], f32, isOutput=False)  # spT[120] | nfT[72]
    w1_d = nc.declare_dram_parameter("w1p", [SL, HIDP], f32, isOutput=False)
    mask_d = nc.declare_dram_parameter("mask8", [BPC, ASL], f32, isOutput=False)
    idx_d = nc.declare_dram_parameter("idx16", [128, 16], i16, isOutput=False)
    outp_d = nc.declare_dram_parameter("out_p", [BPC, ASL], f32, isOutput=True)
    outv_d = nc.declare_dram_parameter("out_v", [1, BPC], f32, isOutput=True)
    fp_d = nc.dram_tensor("fp_scratch", [BOND, PAIRS], f32)

    with tile.TileContext(nc) as tc:
        with (
            tc.tile_pool(name="cpool", bufs=1) as cpool,
            tc.tile_pool(name="w1pool", bufs=4) as w1pool,
            tc.tile_pool(name="ppool", bufs=2, space="PSUM") as ppool,
            tc.tile_pool(name="pab", bufs=1, space="PSUM") as pab,
            tc.tile_pool(name="pshort", bufs=3, space="PSUM") as pshort,
        ):
            # ---- input loads ----
            consts = cpool.tile([128, CF], f32)
            nc.scalar.dma_start(consts[:], consts_d[:])
            acts = cpool.tile([128, 192], f32)
            nc.scalar.dma_start(acts[:], acts_d[:])
            idxs = cpool.tile([128, 16], i16)
            nc.scalar.dma_start(idxs[:], idx_d[:])

            # mask / gather-source tiles (memset first: only rows 16*b are real)
            Xt = cpool.tile([128, ASL], f32, tag="Xt")
            Mt = cpool.tile([128, ASL], f32, tag="Mt")
            nc.vector.memset(Xt[:], 0.0)
            nc.vector.memset(Mt[:], 0.0)
            m_in = mask_d[:]
            m_out = bass.AP(Mt[:].tensor, Mt[:].offset, [[16 * ASL, BPC], [1, ASL]])
            nc.scalar.dma_start(m_out, m_in)

            # ACT warm-up: load the Exp table early so the real Exp is cheap
            warm = cpool.tile([1, 1], f32)
            nc.vector.memset(warm[:], 0.0)
            warmo = cpool.tile([1, 1], f32)
            nc.scalar.activation(warmo[:], warm[:], Act.Exp)

            sp = acts[:, 0:120]    # spT tiled [128, 15*8]
            nf = acts[:, 120:192]  # nfT [128, 72]

            # ---- spec MLP layer 1: h1[hid, b] over 8 hidden chunks ----
            # each k-tile forms complete psum groups (one bank can hold only
            # one open accumulation group); accumulate across k in SBUF on DVE
            h1s = cpool.tile([MCH, 64], f32)
            for k in range(KT):
                kk = 128 if k < KT - 1 else SL - 128 * (KT - 1)  # 9 for last
                w1k = w1pool.tile([128, HIDP], f32, tag="w1k")
                nc.sync.dma_start(w1k[:kk, :], w1_d[128 * k : 128 * k + kk, :])
                h1p = ppool.tile([MCH, 64], f32, tag="h1p")
                for j in range(8):
                    nc.tensor.matmul(
                        h1p[:, 8 * j : 8 * j + 8],
                        w1k[:kk, MCH * j : MCH * (j + 1)],
                        sp[:kk, 8 * k : 8 * k + 8],
                        start=True,
                        stop=True,
                    )
                if k == 0:
                    nc.vector.tensor_copy(h1s[:], h1p[:])
                else:
                    nc.vector.tensor_tensor(h1s[:], h1s[:], h1p[:], op=Alu.add)
            # bias + relu (packed bias tile B1t[p, 8j+b] = b1p[113j+p])
            nc.vector.tensor_tensor(h1s[:], h1s[:], consts[:MCH, OFF_B1T : OFF_B1T + 64], op=Alu.add)
            nc.vector.tensor_scalar_max(h1s[:], h1s[:], 0.0)

            # ---- layer 2: sT[q, b] = relu(W2.T @ h1 + b2) ----
            sps = pshort.tile([SC, BPC], f32, tag="sh")
            for j in range(8):
                nc.tensor.matmul(
                    sps[:],
                    consts[:MCH, OFF_W2T + 100 * j : OFF_W2T + 100 * (j + 1)],
                    h1s[:, 8 * j : 8 * j + 8],
                    start=(j == 0),
                    stop=(j == 7),
                )
            sTs = cpool.tile([SC, BPC], f32)
            nc.vector.tensor_scalar(sTs[:], sps[:], consts[:SC, OFF_B2 : OFF_B2 + 1], 0.0, op0=Alu.add, op1=Alu.max)

            # ---- value head ----
            ro = cpool.tile([128, BPC], f32)  # readoutT = per-graph sum of 9 node cols
            nc.vector.reduce_sum(ro[:], nf.rearrange("p (b n) -> p b n", n=NPG), axis=mybir.AxisListType.X)
            y1 = pshort.tile([64, BPC], f32, tag="sh")
            nc.tensor.matmul(y1[:], consts[:, OFF_WV1 : OFF_WV1 + 64], ro[:], start=True, stop=False)
            nc.tensor.matmul(y1[:], consts[:SC, OFF_WV1 + 64 : OFF_WV1 + 128], sTs[:], start=False, stop=True)
            y1s = cpool.tile([64, BPC], f32)
            nc.vector.tensor_scalar(y1s[:], y1[:], consts[:64, OFF_BV1 : OFF_BV1 + 1], 0.0, op0=Alu.add, op1=Alu.max)
            vps = pshort.tile([1, BPC], f32, tag="sh")
            nc.tensor.matmul(vps[:], consts[:64, OFF_WV2 : OFF_WV2 + 1], y1s[:], start=True, stop=True)
            vs = cpool.tile([1, BPC], f32)
            nc.vector.tensor_scalar_add(vs[:], vps[:], consts[:1, OFF_BV2 : OFF_BV2 + 1])
            nc.sync.dma_start(outv_d[:], vs[:])

            # ---- pair features: hT[c, (b,i,j)] ----
            nfr = cpool.tile([128, NODES], f32)
            nc.vector.tensor_scalar_max(nfr[:], nf, 0.0)
            aips = pab.tile([128, NODES], f32, tag="aips")
            nc.tensor.matmul(aips[:], consts[:, OFF_WA2A : OFF_WA2A + 128], nfr[:], start=True, stop=True)
            bjps = pab.tile([128, NODES], f32, tag="bjps")
            nc.tensor.matmul(bjps[:], consts[:, OFF_WA2B : OFF_WA2B + 128], nfr[:], start=True, stop=True)
            bjs = cpool.tile([128, NODES], f32)
            nc.vector.tensor_copy(bjs[:], bjps[:])
            dps = pshort.tile([128, BPC], f32, tag="sh")
            nc.tensor.matmul(dps[:], consts[:SC, OFF_WA2C : OFF_WA2C + 128], sTs[:], start=True, stop=True)
            dt2 = cpool.tile([128, BPC], f32)
            nc.vector.tensor_scalar_add(dt2[:], dps[:], consts[:, OFF_BA2 : OFF_BA2 + 1])  # + ba2 (per-partition)
            # Ai2[c, (b,i)] = AiT + D'[c,b]
            ai2 = cpool.tile([128, NODES], f32)
            nc.vector.tensor_tensor(
                ai2[:].rearrange("p (b i) -> p b i", i=NPG),
                aips[:].rearrange("p (b i) -> p b i", i=NPG),
                dt2[:].unsqueeze(2).broadcast_to([128, BPC, NPG]),
                op=Alu.add,
            )
            hT = cpool.tile([128, PAIRS], f32)
            nc.vector.tensor_tensor(
                hT[:].rearrange("p (b i j) -> p b i j", i=NPG, j=NPG),
                ai2[:].rearrange("p (b i) -> p b i", i=NPG).unsqueeze(3).broadcast_to([128, BPC, NPG, NPG]),
                bjs[:].rearrange("p (b j) -> p b j", j=NPG).unsqueeze(2).broadcast_to([128, BPC, NPG, NPG]),
                op=Alu.add,
            )
            nc.vector.tensor_scalar_max(hT[:], hT[:], 0.0)

            # ---- saf: fp[t, pair] = Wf.T @ hT + bf ----
            fp1 = pshort.tile([BOND, PAIRS // 2], f32, tag="sh")
            fp2 = pshort.tile([BOND, PAIRS // 2], f32, tag="sh")
            nc.tensor.matmul(fp1[:], consts[:, OFF_WF : OFF_WF + BOND], hT[:, : PAIRS // 2], start=True, stop=True)
            nc.tensor.matmul(fp2[:], consts[:, OFF_WF : OFF_WF + BOND], hT[:, PAIRS // 2 :], start=True, stop=True)
            fps = cpool.tile([BOND, PAIRS], f32)
            nc.vector.tensor_scalar_add(fps[:, : PAIRS // 2], fp1[:], consts[:BOND, OFF_BF : OFF_BF + 1])
            nc.vector.tensor_scalar_add(fps[:, PAIRS // 2 :], fp2[:], consts[:BOND, OFF_BF : OFF_BF + 1])

            # bounce through DRAM to regroup [3, 648] -> X[16b, t*81+e]
            nc.sync.dma_start(fp_d[:], fps[:])
            x_out = bass.AP(Xt[:].tensor, Xt[:].offset, [[16 * ASL, BPC], [1, ASL]])
            x_in = bass.AP(fp_d[:].tensor, 0, [[81, BPC], [PAIRS, BOND], [1, 81]])
            nc.sync.dma_start(x_out, x_in)

            # ---- gather + masked softmax ----
            G = cpool.tile([128, 256], f32)
            nc.gpsimd.ap_gather(G[:], Xt[:], idxs[:], channels=128, num_elems=ASL, d=1, num_idxs=256)
            X2 = cpool.tile([128, ASL], f32)
            nc.vector.tensor_tensor(X2[:], G[:, :ASL], Mt[:], op=Alu.add)
            nmx = cpool.tile([128, 1], f32)
            nc.vector.reduce_max(nmx[:], X2[:], axis=mybir.AxisListType.X, negate=True)
            E = cpool.tile([128, ASL], f32)
            sums = cpool.tile([128, 1], f32)
            nc.scalar.activation(E[:], X2[:], Act.Exp, bias=nmx[:], accum_out=sums[:])
            rc = cpool.tile([128, 1], f32)
            nc.vector.reciprocal(rc[:], sums[:])
            OU = cpool.tile([128, ASL], f32)
            nc.vector.tensor_scalar_mul(OU[:], E[:], rc[:])
            o_in = bass.AP(OU[:].tensor, OU[:].offset, [[16 * ASL, BPC], [1, ASL]])
            nc.sync.dma_start(outp_d[:], o_in)

            if DEBUG_TAPS:
                taps = {
                    "t_acts": acts, "t_con": consts,
                    "t_h1s": h1s, "t_sTs": sTs, "t_ro": ro, "t_y1s": y1s,
                    "t_nfr": nfr, "t_dt2": dt2, "t_ai2": ai2, "t_bjs": bjs,
                    "t_hT": hT, "t_fps": fps, "t_Xt": Xt, "t_G": G,
                    "t_X2": X2, "t_nmx": nmx, "t_E": E, "t_sums": sums,
                }
                for tname, ttile in taps.items():
                    shp = list(ttile[:].shape)
                    td = nc.declare_dram_parameter(tname, shp, f32, isOutput=True)
                    nc.sync.dma_start(td[:], ttile[:])

    nc.compile()
    return nc


def _marshal(node_features, specs, mask, indexmask, W1, b1, W2, b2,
             Wv1, bv1, Wv2, bv2, Wa2, ba2, Wf, bf):
    """Host-side sharding + layout packing. Returns in_maps (one per core)."""
    CF = 585 + 800
    w1p = np.zeros((SL, HIDP), np.float32)
    w1p[:, :HID] = W1

    consts = np.zeros((128, CF), np.float32)
    consts[:, 0:128] = Wa2[0:128]
    consts[:, 128:256] = Wa2[128:256]
    consts[:, 256:320] = Wv1[0:128]          # wv1t cols 0:64
    consts[:100, 320:384] = Wv1[128:228]     # wv1t cols 64:128
    consts[:100, 384:512] = Wa2[256:356]     # wa2c
    consts[:64, 512] = Wv2[:, 0]             # wv2
    consts[:100, 513] = b2
    consts[:64, 514] = bv1
    consts[:1, 515] = bv2
    consts[:, 516] = ba2
    consts[:BOND, 517] = bf
    consts[:, 518:521] = Wf                  # wf [128,3]
    b1p = np.zeros(HIDP, np.float32)
    b1p[:HID] = b1
    consts[:MCH, 521:585] = np.broadcast_to(
        b1p.reshape(8, MCH).T[:, :, None], (MCH, 8, 8)
    ).reshape(MCH, 64)
    w2p = np.zeros((HIDP, SC), np.float32)
    w2p[:HID] = W2
    consts[:MCH, 585:1385] = w2p.reshape(8, MCH, SC).transpose(1, 0, 2).reshape(MCH, 800)

    # index remap to device fp layout (t-major): v -> (v%3)*81 + v//3,
    # then wrap per-graph lists across the 16 partitions of its group
    v = indexmask.astype(np.int64)
    newidx = ((v % BOND) * (NPG * NPG) + v // BOND).astype(np.int16)  # [64, 243]

    in_maps = []
    for c in range(NCORES):
        gsl = slice(c * BPC, (c + 1) * BPC)
        nsl = slice(c * NODES, (c + 1) * NODES)
        spc = np.zeros((BPC, SLP), np.float32)
        spc[:, :SL] = specs[gsl, 0, :]
        spT = spc.reshape(BPC, KT, 128).transpose(2, 1, 0).reshape(128, KT * BPC)
        acts = np.zeros((128, 192), np.float32)
        acts[:, 0:120] = spT
        acts[:, 120:192] = node_features[nsl].T
        padidx = np.zeros((BPC, 256), np.int16)
        padidx[:, :ASL] = newidx[gsl]
        idx16 = padidx.reshape(BPC, 16, 16).transpose(0, 2, 1).reshape(128, 16)
        in_maps.append({
            "consts": consts,
            "acts": acts,
            "w1p": w1p,
            "mask8": np.ascontiguousarray(mask[gsl], np.float32),
            "idx16": np.ascontiguousarray(idx16),
        })
    return in_maps


def _run(inputs, trace=False):
    from concourse.bass_utils import run_bass_kernel_spmd

    if "nc" not in _CACHE:
        _CACHE["nc"] = _build_nc()
    nc = _CACHE["nc"]

    in_maps = _marshal(
        _f32(inputs["node_features"]), _f32(inputs["specs"]),
        _f32(inputs["mask"]), np.asarray(inputs["indexmask"]),
        _f32(inputs["W1"]), _f32(inputs["b1"]), _f32(inputs["W2"]), _f32(inputs["b2"]),
        _f32(inputs["Wv1"]), _f32(inputs["bv1"]), _f32(inputs["Wv2"]), _f32(inputs["bv2"]),
        _f32(inputs["Wa2"]), _f32(inputs["ba2"]), _f32(inputs["Wf"]), _f32(inputs["bf"]),
    )
    res = run_bass_kernel_spmd(nc, in_maps, core_ids=list(range(NCORES)), trace=trace)
    probs = np.concatenate([res.results[c]["out_p"] for c in range(NCORES)], axis=0)
    v = np.concatenate([res.results[c]["out_v"][0] for c in range(NCORES)])[:, None]
    return (probs, v.astype(np.float32)), res


def kernel(**inputs):
    (probs, v), _ = _run(inputs, trace=False)
    return probs, v
